# revision 32
# baseline (speedup 1.0000x reference)
"""Trainium2 Bass kernel for ChunkedMultiHeadCardPassingLayer.

Self-contained: hardcodes shapes (B,T,C)=(4,4096,1024), H=16, CS=128.
Sharding: 8 cores = data-parallel over B (4) x sequence-parallel over T
halves (2). The cross-half carry (prefix sum of chunk sums) is exchanged
with a tiny pair-wise AllReduce mid-kernel.

Matmuls run as float32r (relaxed fp32, 4x faster than fp32 on the PE,
~1.5e-4 rounding vs ~2.3e-3 for bf16).
"""
import os
import sys

sys.path.insert(0, "/opt/trn_rl_repo")

import numpy as np

B, T, C = 4, 4096, 1024
H, D, CS = 16, 64, 128
NCORES = 8
TOK = T // 2            # tokens per core
NCH = TOK // CS         # 16 chunks per core
KK = C // 128           # 8 channel blocks
CPG = 4                 # chunks per MLP group
NGRP = NCH // CPG       # 4 groups
EPS = 1e-5

_compiled = {}


def _build(use_carry_gb, use_card_gb, use_ln_gb, use_mgb=False, use_pb=False, timing_mode=False, phase_limit=4):
    import concourse.bass as bass
    import concourse.bacc as bacc
    import concourse.tile as tile
    from concourse import mybir
    from contextlib import ExitStack

    f32 = mybir.dt.float32
    f32r = mybir.dt.float32r
    AF = mybir.ActivationFunctionType
    OP = mybir.AluOpType

    nc = bacc.Bacc("TRN2", target_bir_lowering=False, debug=False,
                   enable_asserts=False, num_devices=NCORES)

    # ---------------- DRAM I/O ----------------
    x_d = nc.dram_tensor("x", [TOK, C], f32r, kind="ExternalInput")
    wm_d = nc.dram_tensor("wm", [C, C], f32r, kind="ExternalInput")
    wg_d = nc.dram_tensor("wg", [C, C], f32r, kind="ExternalInput")
    wp_d = nc.dram_tensor("wp", [C, C], f32r, kind="ExternalInput")
    mbrow_d = nc.dram_tensor("mbrow", [1, C], f32r, kind="ExternalInput")
    gbrow_d = nc.dram_tensor("gbrow", [1, C], f32r, kind="ExternalInput")
    pbrow_d = nc.dram_tensor("pbrow", [1, C], f32r, kind="ExternalInput")
    w1x2_d = nc.dram_tensor("w1x2", [128, 2, 128], f32r, kind="ExternalInput")
    w2_d = nc.dram_tensor("w2", [128, 64], f32r, kind="ExternalInput")
    b1col_d = nc.dram_tensor("b1col", [128, 1], f32, kind="ExternalInput")
    b2col_d = nc.dram_tensor("b2col", [128, 1], f32, kind="ExternalInput")
    uincl_d = nc.dram_tensor("uincl", [128, 128], f32r, kind="ExternalInput")
    ident_d = nc.dram_tensor("ident", [128, 128], f32r, kind="ExternalInput")
    sel_d = nc.dram_tensor("sel", [128, NCH, 16], f32r, kind="ExternalInput")
    rowsel_d = nc.dram_tensor("rowsel", [16, NCH, 128], f32r, kind="ExternalInput")
    u16_d = nc.dram_tensor("u16", [16, 16], f32r, kind="ExternalInput")
    ones16c_d = nc.dram_tensor("ones16c", [16, 1], f32r, kind="ExternalInput")
    ones1x16_d = nc.dram_tensor("ones1x16", [1, 16], f32r, kind="ExternalInput")
    ones1x128_d = nc.dram_tensor("ones1x128", [1, 128], f32r, kind="ExternalInput")
    hmask_d = nc.dram_tensor("hmask", [1, 1], f32, kind="ExternalInput")
    imask_d = nc.dram_tensor("imask", [1, 1], f32, kind="ExternalInput")
    if use_carry_gb:
        gcarry_d = nc.dram_tensor("gcarry", [1, C], f32, kind="ExternalInput")
        bcarry_d = nc.dram_tensor("bcarry", [1, C], f32, kind="ExternalInput")
    if use_card_gb:
        gcard_d = nc.dram_tensor("gcard", [1, C], f32, kind="ExternalInput")
        bcard_d = nc.dram_tensor("bcard", [1, C], f32, kind="ExternalInput")
    if use_ln_gb:
        gln_d = nc.dram_tensor("gln", [1, C], f32, kind="ExternalInput")
        bln_d = nc.dram_tensor("bln", [1, C], f32, kind="ExternalInput")
    out_d = nc.dram_tensor("out", [TOK, C], f32, kind="ExternalOutput")

    def bcast_dram(handle, parts):
        a = handle.ap()
        return bass.AP(tensor=a.tensor, offset=a.offset,
                       ap=[[0, parts]] + list(a.ap)[1:])

    def bc_seg(ap2d, reps):
        """(P, G) -> (P, G, reps) broadcast along a new inner dim."""
        return bass.AP(tensor=ap2d.tensor, offset=ap2d.offset,
                       ap=[list(ap2d.ap)[0], list(ap2d.ap)[1], [0, reps]])

    with tile.TileContext(nc, pool_alloc_mode="queue") as tc:
        es = ExitStack()
        with es:
            consts = es.enter_context(tc.tile_pool(name="consts", bufs=1))

            # --------- load constants ---------
            uincl = consts.tile([128, 128], f32r)
            ident = consts.tile([128, 128], f32r)
            sel = consts.tile([128, NCH, 16], f32r)
            u16 = consts.tile([16, 16], f32r)
            ones16c = consts.tile([16, 1], f32r)
            ones1x16 = consts.tile([1, 16], f32r)
            ones1x128 = consts.tile([1, 128], f32r)
            w1x2 = consts.tile([128, 2, 128], f32r)
            w2 = consts.tile([128, 64], f32r)
            b1col = consts.tile([128, 1], f32)
            b2col = consts.tile([128, 1], f32)
            hmask = consts.tile([1, 1], f32)
            imask = consts.tile([1, 1], f32)
            epscol = consts.tile([128, 1], f32)
            # ident feeds the very first PE transposes -> sync ring, first.
            # Everything else rides the ACT HWDGE ring so the ~1.3us fixed
            # cost per small DMA doesn't queue ahead of the x chunk loads.
            nc.sync.dma_start(ident[:], ident_d.ap())
            for t_, d_ in ((uincl, uincl_d), (sel, sel_d),
                           (u16, u16_d), (ones16c, ones16c_d),
                           (ones1x16, ones1x16_d), (ones1x128, ones1x128_d),
                           (w1x2, w1x2_d), (w2, w2_d), (b1col, b1col_d),
                           (b2col, b2col_d), (hmask, hmask_d), (imask, imask_d)):
                nc.scalar.dma_start(t_[:], d_.ap())
            nc.vector.memset(epscol[:], EPS)

            p_gated = tc.alloc_tile_pool(name="gated", bufs=NCH, side="right")
            p_dram = es.enter_context(tc.tile_pool(name="dram", bufs=1, space="DRAM"))
            gated_tiles = []

            # ================= Phase 1: marks/gates =================
            with nc.named_scope("p1_markgate"):
                with (
                    tc.tile_pool(name="wmg", bufs=1) as p_wmg,
                    tc.tile_pool(name="xn1", bufs=3) as p_xn,
                    tc.tile_pool(name="xt1", bufs=3) as p_xt,
                    tc.tile_pool(name="gts", bufs=2) as p_gts,
                    tc.tile_pool(name="mg_ps", bufs=6, space="PSUM") as p_mg,
                    tc.tile_pool(name="xtp_ps", bufs=2, space="PSUM") as p_xtp,
                ):
                    wm = p_wmg.tile([128, KK, C], f32r)
                    wg = p_wmg.tile([128, KK, C], f32r)
                    mbrow = p_wmg.tile([1, C], f32r)
                    gbrow = p_wmg.tile([1, C], f32r)
                    if use_mgb:
                        nc.sync.dma_start(mbrow[:], mbrow_d.ap())
                        nc.sync.dma_start(gbrow[:], gbrow_d.ap())
                    wm_r = wm_d.ap().rearrange("(k p) n -> k p n", p=128)
                    wg_r = wg_d.ap().rearrange("(k p) n -> k p n", p=128)

                    def emit_xpose(i):
                        xn = p_xn.tile([128, C], f32r, name=f"xn_{i}", tag="xn")
                        nc.sync.dma_start(xn[:], x_d.ap()[i * CS:(i + 1) * CS, :])
                        xtp0 = p_xtp.tile([128, 512], f32r, name=f"xtp0_{i}", tag="xtp")
                        xtp1 = p_xtp.tile([128, 512], f32r, name=f"xtp1_{i}", tag="xtp")
                        for k in range(KK):
                            dst = xtp0 if k < 4 else xtp1
                            nc.tensor.transpose(
                                dst[:, (k % 4) * CS:(k % 4 + 1) * CS],
                                xn[:, k * CS:(k + 1) * CS], ident[:])
                        xt = p_xt.tile([128, KK, CS], f32r, name=f"xt_{i}", tag="xt")
                        nc.scalar.copy(xt[:, 0:4, :],
                                       xtp0[:].rearrange("p (a b) -> p a b", a=4))
                        nc.scalar.copy(xt[:, 4:8, :],
                                       xtp1[:].rearrange("p (a b) -> p a b", a=4))
                        return xt

                    xt_next = emit_xpose(0)
                    # weight preload AFTER the first chunk's load/transpose so
                    # the 8MB of weight DMA doesn't delay PE startup
                    for k in range(KK):
                        nc.gpsimd.dma_start(wm[:, k, :], wm_r[k])
                        nc.gpsimd.dma_start(wg[:, k, :], wg_r[k])
                    for i in range(NCH):
                        xt = xt_next
                        if i + 1 < NCH:
                            xt_next = emit_xpose(i + 1)

                        mgh = [p_mg.tile([128, 512], f32,
                                          name=f"mg_{i}_{t}", tag="mg")
                               for t in range(4)]  # m_lo, m_hi, g_lo, g_hi
                        for k in range(KK):
                            for nb in range(2):
                                ns = slice(nb * 512, (nb + 1) * 512)
                                last = (k == KK - 1) and not use_mgb
                                nc.tensor.matmul(mgh[nb][:], xt[:, k, :],
                                                 wm[:, k, ns],
                                                 start=(k == 0), stop=last)
                                nc.tensor.matmul(mgh[2 + nb][:], xt[:, k, :],
                                                 wg[:, k, ns],
                                                 start=(k == 0), stop=last)
                        if use_mgb:
                            for nb in range(2):
                                ns = slice(nb * 512, (nb + 1) * 512)
                                nc.tensor.matmul(mgh[nb][:], ones1x128[:],
                                                 mbrow[:, ns], start=False,
                                                 stop=True)
                                nc.tensor.matmul(mgh[2 + nb][:], ones1x128[:],
                                                 gbrow[:, ns], start=False,
                                                 stop=True)

                        gts = p_gts.tile([128, C], f32, name=f"gts_{i}", tag="gts")
                        gated = p_gated.tile([128, C], f32r, name=f"gated_{i}",
                                             tag="gated")
                        for nb in range(2):
                            ns = slice(nb * 512, (nb + 1) * 512)
                            nc.scalar.activation(gts[:, ns], mgh[2 + nb][:],
                                                 AF.Sigmoid)
                            nc.vector.tensor_tensor(out=gated[:, ns],
                                                    in0=mgh[nb][:],
                                                    in1=gts[:, ns], op=OP.mult)
                        gated_tiles.append(gated)

            # ============ Phase 1.5: chunk sums S ============
            with nc.named_scope("p15_chunksums"):
                with tc.tile_pool(name="s_ps", bufs=1, space="PSUM") as p_sps:
                    s_ps = p_sps.tile([16, C], f32)
                    for i in range(NCH):
                        for nb in range(2):
                            ns = slice(nb * 512, (nb + 1) * 512)
                            nc.tensor.matmul(s_ps[:, ns], sel[:, i, :],
                                             gated_tiles[i][:, ns],
                                             start=(i == 0), stop=(i == NCH - 1))
                    s_sb = consts.tile([16, C], f32r, name="s_sb")
                    nc.vector.tensor_copy(s_sb[:], s_ps[:])

            # ============ Interlude: carry exchange + carry LN ============
            p_ip = tc.alloc_tile_pool(name="ip_ps", bufs=3, space="PSUM")
            p_ctp = tc.alloc_tile_pool(name="ctp_ps", bufs=2, space="PSUM")
            with nc.named_scope("p_carry"):
                with (
                    tc.tile_pool(name="car_sb", bufs=1) as p_csb,
                ):
                    if use_carry_gb:
                        gcarry = p_csb.tile([16, C], f32)
                        bcarry = p_csb.tile([16, C], f32)
                        nc.gpsimd.dma_start(gcarry[:], bcast_dram(gcarry_d, 16))
                        nc.gpsimd.dma_start(bcarry[:], bcast_dram(bcarry_d, 16))
                    tot_ps = p_ip.tile([1, C], f32, name="tot_ps", tag="ip")
                    for nb in range(2):
                        ns = slice(nb * 512, (nb + 1) * 512)
                        nc.tensor.matmul(tot_ps[:, ns], ones16c[:], s_sb[:, ns],
                                         start=True, stop=True)
                    masked = p_csb.tile([1, C], f32)
                    nc.vector.tensor_scalar_mul(out=masked[:], in0=tot_ps[:],
                                                scalar1=imask[:])
                    ccin = p_dram.tile([1, C], f32)
                    ccout = p_dram.tile([1, C], f32)
                    nc.sync.dma_start(ccin[:], masked[:])
                    if timing_mode:
                        nc.sync.dma_start(ccout[:], ccin[:])
                    else:
                        nc.gpsimd.collective_compute(
                            "AllReduce", OP.add,
                            replica_groups=[[0, 1], [2, 3], [4, 5], [6, 7]],
                            ins=[ccin.opt()], outs=[ccout.opt()])
                    agg = p_csb.tile([1, C], f32)
                    nc.sync.dma_start(agg[:], ccout[:])
                    carry_in = p_csb.tile([1, C], f32r)
                    nc.vector.tensor_scalar_mul(out=carry_in[:], in0=agg[:],
                                                scalar1=hmask[:])

                    carr_ps = p_ip.tile([16, C], f32, name="carr_ps", tag="ip")
                    for nb in range(2):
                        ns = slice(nb * 512, (nb + 1) * 512)
                        nc.tensor.matmul(carr_ps[:, ns], u16[:], s_sb[:, ns],
                                         start=True, stop=False)
                        nc.tensor.matmul(carr_ps[:, ns], ones1x16[:],
                                         carry_in[:, ns], start=False, stop=True)

                    # carry LN over d=64 per head, 16 chunks on partitions
                    csums = p_csb.tile([16, 16], f32)
                    nc.vector.reduce_sum(
                        out=csums[:],
                        in_=carr_ps[:].rearrange("p (g d) -> p g d", g=16),
                        axis=mybir.AxisListType.X)
                    csq = p_csb.tile([16, C], f32)
                    nc.scalar.square(csq[:], carr_ps[:])
                    csqs = p_csb.tile([16, 16], f32)
                    nc.vector.reduce_sum(
                        out=csqs[:],
                        in_=csq[:].rearrange("p (g d) -> p g d", g=16),
                        axis=mybir.AxisListType.X)
                    cmean = p_csb.tile([16, 16], f32)
                    nc.vector.tensor_scalar_mul(out=cmean[:], in0=csums[:],
                                                scalar1=1.0 / D)
                    cvar = p_csb.tile([16, 16], f32)
                    nc.vector.tensor_scalar_mul(out=cvar[:], in0=csqs[:],
                                                scalar1=1.0 / D)
                    cmsq = p_csb.tile([16, 16], f32)
                    nc.vector.tensor_tensor(out=cmsq[:], in0=cmean[:],
                                            in1=cmean[:], op=OP.mult)
                    nc.vector.tensor_tensor(out=cvar[:], in0=cvar[:],
                                            in1=cmsq[:], op=OP.subtract)
                    crstd = p_csb.tile([16, 16], f32)
                    nc.scalar.activation(crstd[:], cvar[:], AF.Sqrt,
                                         bias=epscol[0:16])
                    nc.vector.reciprocal(out=crstd[:], in_=crstd[:])

                    t1c = p_csb.tile([16, C], f32)
                    nc.vector.tensor_tensor(
                        out=t1c[:].rearrange("p (g d) -> p g d", g=16),
                        in0=carr_ps[:].rearrange("p (g d) -> p g d", g=16),
                        in1=bc_seg(cmean[:], D), op=OP.subtract)
                    ncr = consts.tile([16, C], f32r, name="ncr")
                    if use_carry_gb:
                        nc.vector.tensor_tensor(
                            out=t1c[:].rearrange("p (g d) -> p g d", g=16),
                            in0=t1c[:].rearrange("p (g d) -> p g d", g=16),
                            in1=bc_seg(crstd[:], D), op=OP.mult)
                        nc.vector.tensor_tensor(out=t1c[:], in0=t1c[:],
                                                in1=gcarry[:], op=OP.mult)
                        nc.vector.tensor_tensor(out=ncr[:], in0=t1c[:],
                                                in1=bcarry[:], op=OP.add)
                    else:
                        nc.vector.tensor_tensor(
                            out=ncr[:].rearrange("p (g d) -> p g d", g=16),
                            in0=t1c[:].rearrange("p (g d) -> p g d", g=16),
                            in1=bc_seg(crstd[:], D), op=OP.mult)

            # ============ Phase 2a: cards ============
            p_ctg = tc.alloc_tile_pool(name="ctg", bufs=NGRP)
            ctg = [p_ctg.tile([128, KK, CPG * CS], f32r, name=f"ctg_{g}",
                              tag="ctg") for g in range(NGRP)]
            with nc.named_scope("p2a_cards"):
              if phase_limit >= 2:
                with (
                    tc.tile_pool(name="rsel", bufs=1) as p_rsel,
                    tc.tile_pool(name="cl2", bufs=(3 if use_card_gb else 4)) as p_cl,
                    tc.tile_pool(name="sq2", bufs=(3 if use_card_gb else 4)) as p_sq,
                    tc.tile_pool(name="cards2", bufs=(2 if use_card_gb else 4)) as p_cards,
                    tc.tile_pool(name="st2", bufs=(6 if use_card_gb else 8)) as p_st2,
                ):
                    rowsel = p_rsel.tile([16, NCH, 128], f32r)
                    nc.sync.dma_start(rowsel[:], rowsel_d.ap())
                    if use_card_gb:
                        gcard = p_rsel.tile([128, C], f32)
                        bcard = p_rsel.tile([128, C], f32)
                        nc.gpsimd.dma_start(gcard[:], bcast_dram(gcard_d, 128))
                        nc.gpsimd.dma_start(bcard[:], bcast_dram(bcard_d, 128))
                    for i in range(NCH):
                        g, ti = i // CPG, i % CPG
                        ip = p_ip.tile([128, C], f32, name=f"ip_{i}", tag="ip")
                        for nb in range(2):
                            ns = slice(nb * 512, (nb + 1) * 512)
                            nc.tensor.matmul(ip[:, ns], uincl[:],
                                             gated_tiles[i][:, ns],
                                             start=True, stop=False)
                            nc.tensor.matmul(ip[:, ns], rowsel[:, i, :],
                                             ncr[:, ns], start=False, stop=True)
                        cl = p_cl.tile([128, C], f32, name=f"cl_{i}", tag="cl")
                        nc.vector.tensor_tensor(out=cl[:], in0=ip[:],
                                                in1=gated_tiles[i][:],
                                                op=OP.subtract)
                        sums = p_st2.tile([128, 16], f32, name=f"sums_{i}", tag="sums")
                        nc.vector.reduce_sum(
                            out=sums[:],
                            in_=cl[:].rearrange("p (g d) -> p g d", g=16),
                            axis=mybir.AxisListType.X)
                        sq = p_sq.tile([128, C], f32, name=f"sq_{i}", tag="sq")
                        nc.scalar.square(sq[:], cl[:])
                        sqs = p_st2.tile([128, 16], f32, name=f"sqs_{i}", tag="sqs")
                        nc.vector.reduce_sum(
                            out=sqs[:],
                            in_=sq[:].rearrange("p (g d) -> p g d", g=16),
                            axis=mybir.AxisListType.X)
                        mean = p_st2.tile([128, 16], f32, name=f"mean_{i}", tag="mean")
                        nc.vector.tensor_scalar_mul(out=mean[:], in0=sums[:],
                                                    scalar1=1.0 / D)
                        var = p_st2.tile([128, 16], f32, name=f"var_{i}", tag="var")
                        nc.vector.tensor_scalar_mul(out=var[:], in0=sqs[:],
                                                    scalar1=1.0 / D)
                        msq = p_st2.tile([128, 16], f32, name=f"msq_{i}", tag="msq")
                        nc.vector.tensor_tensor(out=msq[:], in0=mean[:],
                                                in1=mean[:], op=OP.mult)
                        nc.vector.tensor_tensor(out=var[:], in0=var[:],
                                                in1=msq[:], op=OP.subtract)
                        rstd = p_st2.tile([128, 16], f32, name=f"rstd_{i}", tag="rstd")
                        nc.scalar.activation(rstd[:], var[:], AF.Sqrt,
                                             bias=epscol[:])
                        nc.vector.reciprocal(out=rstd[:], in_=rstd[:])

                        nc.gpsimd.tensor_tensor(
                            out=cl[:].rearrange("p (g d) -> p g d", g=16),
                            in0=cl[:].rearrange("p (g d) -> p g d", g=16),
                            in1=bc_seg(mean[:], D), op=OP.subtract)
                        cards = p_cards.tile([128, C], f32r, name=f"cards_{i}",
                                             tag="cards")
                        if use_card_gb:
                            nc.vector.tensor_tensor(
                                out=cl[:].rearrange("p (g d) -> p g d", g=16),
                                in0=cl[:].rearrange("p (g d) -> p g d", g=16),
                                in1=bc_seg(rstd[:], D), op=OP.mult)
                            nc.vector.tensor_tensor(out=cl[:], in0=cl[:],
                                                    in1=gcard[:], op=OP.mult)
                            nc.vector.tensor_tensor(out=cards[:], in0=cl[:],
                                                    in1=bcard[:], op=OP.add)
                        else:
                            nc.gpsimd.tensor_tensor(
                                out=cards[:].rearrange("p (g d) -> p g d", g=16),
                                in0=cl[:].rearrange("p (g d) -> p g d", g=16),
                                in1=bc_seg(rstd[:], D), op=OP.mult)

                        ctp0 = p_ctp.tile([128, 512], f32r, name=f"ctp0_{i}", tag="ctp")
                        ctp1 = p_ctp.tile([128, 512], f32r, name=f"ctp1_{i}", tag="ctp")
                        for k in range(KK):
                            dst = ctp0 if k < 4 else ctp1
                            nc.tensor.transpose(
                                dst[:, (k % 4) * CS:(k % 4 + 1) * CS],
                                cards[:, k * CS:(k + 1) * CS], ident[:])
                        nc.scalar.copy(
                            ctg[g][:, 0:4, ti * CS:(ti + 1) * CS],
                            ctp0[:].rearrange("p (a b) -> p a b", a=4))
                        nc.scalar.copy(
                            ctg[g][:, 4:8, ti * CS:(ti + 1) * CS],
                            ctp1[:].rearrange("p (a b) -> p a b", a=4))

            p_ctp.release()
            p_ip.release()
            p_gated.release()

            # ============ Phase 2b: head MLP ============
            p_hog = tc.alloc_tile_pool(name="hog", bufs=NGRP, side="right")
            hog = [p_hog.tile([128, KK, CPG * CS], f32r, name=f"hog_{g}",
                              tag="hog") for g in range(NGRP)]
            with nc.named_scope("p2b_mlp"):
              if phase_limit >= 3:
                with (
                    tc.tile_pool(name="xn2", bufs=4) as p_xn2,
                    tc.tile_pool(name="xtg2", bufs=1) as p_xtg,
                    tc.tile_pool(name="hs2", bufs=3) as p_hs,
                    tc.tile_pool(name="xtp2_ps", bufs=2, space="PSUM") as p_xtp2,
                    tc.tile_pool(name="hp_ps", bufs=3, space="PSUM") as p_hp,
                    tc.tile_pool(name="hop_ps", bufs=3, space="PSUM") as p_hop,
                ):
                    for g in range(NGRP):
                        xng = []
                        for j in range(CPG):
                            cidx = g * CPG + j
                            xn2 = p_xn2.tile([128, C], f32r, name=f"xn2_{cidx}",
                                             tag="xn2")
                            nc.sync.dma_start(
                                xn2[:], x_d.ap()[cidx * CS:(cidx + 1) * CS, :])
                            xng.append(xn2)
                        xtg = p_xtg.tile([128, KK, CPG * CS], f32r,
                                         name=f"xtg_{g}", tag="xtg")
                        for k in range(KK):
                            xtpk = p_xtp2.tile([128, 512], f32r,
                                               name=f"xtpk_{g}_{k}", tag="xtpk")
                            for j in range(CPG):
                                nc.tensor.transpose(
                                    xtpk[:, j * CS:(j + 1) * CS],
                                    xng[j][:, k * CS:(k + 1) * CS], ident[:])
                            nc.vector.tensor_copy(xtg[:, k, :], xtpk[:])

                        for h in range(H):
                            k, off = h // 2, 64 * (h % 2)
                            po = slice(off, off + 64)
                            hp = p_hp.tile([128, 512], f32, name=f"hp_{g}_{h}",
                                           tag="hp")
                            nc.tensor.matmul(hp[:], w1x2[po, 0, :],
                                             xtg[po, k, :], start=True, stop=False)
                            nc.tensor.matmul(hp[:], w1x2[po, 1, :],
                                             ctg[g][po, k, :], start=False, stop=True)
                            hs = p_hs.tile([128, 512], f32r, name=f"hs_{g}_{h}",
                                           tag="hs")
                            nc.scalar.activation(hs[:], hp[:], AF.Gelu,
                                                 bias=b1col[:])
                            hop = p_hop.tile([64, 512], f32,
                                             name=f"hop_{g}_{h}", tag="hop")
                            nc.tensor.matmul(hop[:], w2[:], hs[:],
                                             start=True, stop=True)
                            nc.vector.tensor_scalar_add(
                                out=hog[g][off:off + 64, k, :], in0=hop[:],
                                scalar1=b2col[0:64])
            p_ctg.release()

            # ============ Phase 3: proj + final LN + residual ============
            with nc.named_scope("p3_proj"):
              if phase_limit >= 4:
                with (
                    tc.tile_pool(name="wp", bufs=1) as p_wp,
                    tc.tile_pool(name="xres", bufs=3) as p_xres,
                    tc.tile_pool(name="t3", bufs=3) as p_t3,
                    tc.tile_pool(name="st3", bufs=3) as p_st3,
                    tc.tile_pool(name="yp_ps", bufs=4, space="PSUM") as p_yp,
                ):
                    wp = p_wp.tile([128, KK, C], f32r)
                    wp_r = wp_d.ap().rearrange("(k p) n -> k p n", p=128)
                    for k in range(KK):
                        nc.scalar.dma_start(wp[:, k, :], wp_r[k])
                    pbrow = p_wp.tile([1, C], f32r)
                    if use_pb:
                        nc.sync.dma_start(pbrow[:], pbrow_d.ap())
                    if use_ln_gb:
                        gln = p_wp.tile([128, C], f32)
                        bln = p_wp.tile([128, C], f32)
                        nc.gpsimd.dma_start(gln[:], bcast_dram(gln_d, 128))
                        nc.gpsimd.dma_start(bln[:], bcast_dram(bln_d, 128))

                    for c in range(NCH):
                        g, ti = c // CPG, c % CPG
                        yp = p_yp.tile([128, C], f32, name=f"yp_{c}", tag="yp")
                        for k in range(KK):
                            for nb in range(2):
                                ns = slice(nb * 512, (nb + 1) * 512)
                                nc.tensor.matmul(
                                    yp[:, ns],
                                    hog[g][:, k, ti * CS:(ti + 1) * CS],
                                    wp[:, k, ns], start=(k == 0),
                                    stop=(k == KK - 1) and not use_pb)
                        if use_pb:
                            for nb in range(2):
                                ns = slice(nb * 512, (nb + 1) * 512)
                                nc.tensor.matmul(yp[:, ns], ones1x128[:],
                                                 pbrow[:, ns], start=False,
                                                 stop=True)

                        xres = p_xres.tile([128, C], f32r, name=f"xres_{c}",
                                           tag="xres")
                        nc.sync.dma_start(xres[:],
                                          x_d.ap()[c * CS:(c + 1) * CS, :])
                        st = p_st3.tile([128, 2, 6], f32, name=f"st_{c}", tag="st")
                        nc.vector.bn_stats(out=st[:, 0, :], in_=yp[:, 0:512])
                        nc.vector.bn_stats(out=st[:, 1, :], in_=yp[:, 512:1024])
                        mv = p_st3.tile([128, 2], f32, name=f"mv_{c}", tag="mv")
                        nc.vector.bn_aggr(out=mv[:], in_=st[:])
                        rstd3 = p_st3.tile([128, 1], f32, name=f"rstd3_{c}",
                                           tag="rstd3")
                        nc.scalar.activation(rstd3[:], mv[:, 1:2], AF.Sqrt,
                                             bias=epscol[:])
                        nc.vector.reciprocal(out=rstd3[:], in_=rstd3[:])
                        t3 = p_t3.tile([128, C], f32, name=f"t3_{c}", tag="t3")
                        nc.vector.tensor_scalar(out=t3[:], in0=yp[:],
                                                scalar1=mv[:, 0:1],
                                                scalar2=rstd3[:],
                                                op0=OP.subtract, op1=OP.mult)
                        if use_ln_gb:
                            nc.vector.tensor_tensor(out=t3[:], in0=t3[:],
                                                    in1=gln[:], op=OP.mult)
                            nc.vector.tensor_tensor(out=t3[:], in0=t3[:],
                                                    in1=bln[:], op=OP.add)
                        nc.gpsimd.tensor_tensor(out=t3[:], in0=t3[:],
                                                in1=xres[:].bitcast(f32),
                                                op=OP.add)
                        nc.sync.dma_start(
                            out_d.ap()[c * CS:(c + 1) * CS, :], t3[:])
            p_hog.release()

    nc.compile()
    return nc


def _get_compiled(flags):
    if flags not in _compiled:
        _compiled[flags] = _build(*flags)
    return _compiled[flags]


def kernel(**inputs):
    f = lambda k: np.ascontiguousarray(np.asarray(inputs[k], np.float32))
    x = f("x")
    mark_W, mark_b = f("mark_W"), f("mark_b")
    gate_W, gate_b = f("gate_W"), f("gate_b")
    carry_g, carry_b = f("carry_g"), f("carry_b")
    card_g, card_b = f("card_g"), f("card_b")
    W1, b1 = f("W1"), f("b1")
    W2, b2 = f("W2"), f("b2")
    proj_W, proj_b = f("proj_W"), f("proj_b")
    ln_g, ln_b = f("ln_g"), f("ln_b")

    use_carry_gb = not (np.all(carry_g == 1.0) and np.all(carry_b == 0.0))
    use_card_gb = not (np.all(card_g == 1.0) and np.all(card_b == 0.0))
    use_ln_gb = not (np.all(ln_g == 1.0) and np.all(ln_b == 0.0))
    use_mgb = not (np.all(mark_b == 0.0) and np.all(gate_b == 0.0))
    use_pb = not np.all(proj_b == 0.0)
    flags = (use_carry_gb, use_card_gb, use_ln_gb, use_mgb, use_pb)
    nc = _get_compiled(flags)

    # host-prepped constants
    w1x2 = np.empty((128, 2, 128), np.float32)
    w1x2[0:64, 0, :] = W1[0:64, :]
    w1x2[64:128, 0, :] = W1[0:64, :]
    w1x2[0:64, 1, :] = W1[64:128, :]
    w1x2[64:128, 1, :] = W1[64:128, :]
    sel = np.zeros((128, NCH, 16), np.float32)
    sel[:, np.arange(NCH), np.arange(NCH)] = 1.0
    rowsel = np.zeros((16, NCH, 128), np.float32)
    rowsel[np.arange(NCH), np.arange(NCH), :] = 1.0
    common = {
        "wm": mark_W, "wg": gate_W, "wp": proj_W,
        "mbrow": mark_b[None, :], "gbrow": gate_b[None, :],
        "pbrow": proj_b[None, :],
        "w1x2": w1x2, "w2": W2,
        "b1col": b1[:, None],
        "b2col": np.concatenate([b2, b2])[:, None],
        "uincl": np.triu(np.ones((128, 128), np.float32)),
        "ident": np.eye(128, dtype=np.float32),
        "sel": sel, "rowsel": rowsel,
        "u16": np.triu(np.ones((16, 16), np.float32), 1),
        "ones16c": np.ones((16, 1), np.float32),
        "ones1x16": np.ones((1, 16), np.float32),
        "ones1x128": np.ones((1, 128), np.float32),
    }
    if use_carry_gb:
        common["gcarry"] = np.tile(carry_g, H)[None, :]
        common["bcarry"] = np.tile(carry_b, H)[None, :]
    if use_card_gb:
        common["gcard"] = np.tile(card_g, H)[None, :]
        common["bcard"] = np.tile(card_b, H)[None, :]
    if use_ln_gb:
        common["gln"] = ln_g[None, :]
        common["bln"] = ln_b[None, :]

    in_maps = []
    for core in range(NCORES):
        b, s = core // 2, core % 2
        m = dict(common)
        m["x"] = np.ascontiguousarray(x[b, s * TOK:(s + 1) * TOK, :])
        m["hmask"] = np.full((1, 1), float(s), np.float32)
        m["imask"] = np.full((1, 1), float(1 - s), np.float32)
        in_maps.append(m)

    kernel.last_in_maps = in_maps
    globals()["_last_in_maps"] = in_maps
    from concourse.bass_utils import run_bass_kernel_spmd
    res = run_bass_kernel_spmd(nc, in_maps, core_ids=list(range(NCORES)))

    out = np.empty((B, T, C), np.float32)
    for core in range(NCORES):
        b, s = core // 2, core % 2
        out[b, s * TOK:(s + 1) * TOK, :] = res.results[core]["out"]
    return out
